# revision 18
# baseline (speedup 1.0000x reference)
"""PhiHarmonicAttention (B=1, S=2048, D=2048, H=16, Dh=128) on 8 Trainium2 cores.

Sharding: tensor-parallel over heads - 2 heads per core.
  - Wq/Wk/Wv column-sliced (256 cols per core), Wo row-sliced (256 rows).
  - Each core computes q^T/k^T (RoPE'd) + v for its 2 heads, causal
    softmax(QK^T)V in transposed layout, and a partial x-out product with its
    Wo slice. Host sums the 8 partials (TP row-parallel reduction).

All matmuls run in bf16 (fp32 PSUM accumulate): ~0.3% rms per GEMM, well
inside the harness 2e-2 gate, and halves HBM traffic vs f32r. fp8 was
measured (in CoreSim) at ~8% output error - attention is a weighted mean
of zero-mean vectors, so per-element quantization noise does NOT average
away relative to the signal - hence bf16 everywhere.

Pipeline: per 512-wide seq chunk ci - projections A(ci), then attention
B(h0,ci), B(h1,ci), then output projection C(ci). Causality means B(*,ci)
only needs A(0..ci), so PE never starves at stage boundaries.

Engine budget: PE does all GEMMs; ACT does exp (+half the out copies);
DVE does rope muls, masking, softmax normalize (+half the out copies);
Pool (gpsimd) takes the PSUM drains (rope raw copy, v copy) and the
denominator partition-broadcast.
"""
import numpy as np
import ml_dtypes
from contextlib import ExitStack, nullcontext

import concourse.bass as bass
import concourse.tile as tile
from concourse import bacc, mybir
from concourse.bass_utils import run_bass_kernel_spmd

S = 2048
D = 2048
H = 16
DH = 128
NCORES = 8
HPC = H // NCORES          # heads per core = 2
CW = HPC * DH              # weight col-slice per core = 256
NO = D // 128              # contraction chunks = 16
W = 512                    # seq chunk width
NCH = S // W               # seq chunks = 4
NB = S // 128              # seq blocks = 16
SCALE = float(1.0 / np.sqrt(np.float32(DH)))

ROT_FACTOR = (1.0 + 5.0 ** 0.5) / 2.0 - 1.0
ROPE_BASE = 10000.0

F32 = mybir.dt.float32
BF16 = mybir.dt.bfloat16
F16 = mybir.dt.float16


def _build_nc(reps=1, stages="ABC", pyreps=1):
    nc = bacc.Bacc("TRN2", target_bir_lowering=False, debug=False, num_devices=NCORES)

    xt_d = nc.dram_tensor("xt16", [D, S], BF16, kind="ExternalInput").ap()
    wq_d = nc.dram_tensor("wq16", [D, CW], BF16, kind="ExternalInput").ap()
    wk_d = nc.dram_tensor("wk16", [D, CW], BF16, kind="ExternalInput").ap()
    wv_d = nc.dram_tensor("wv16", [D, CW], BF16, kind="ExternalInput").ap()
    wo_d = nc.dram_tensor("wo16", [CW, D], BF16, kind="ExternalInput").ap()
    rcu_d = nc.dram_tensor("ropecu", [DH, W], F32, kind="ExternalInput").ap()
    rsu_d = nc.dram_tensor("ropesu", [DH, W], F32, kind="ExternalInput").ap()
    rc512_d = nc.dram_tensor("ropec512", [DH, NCH], F32, kind="ExternalInput").ap()
    rs512_d = nc.dram_tensor("ropes512", [DH, NCH], F32, kind="ExternalInput").ap()
    out_d = nc.dram_tensor("out", [S, D], F16, kind="ExternalOutput").ap()

    with ExitStack() as ctx:
        tc = ctx.enter_context(tile.TileContext(nc))
        consts = ctx.enter_context(tc.tile_pool(name="consts", bufs=1))
        persist = ctx.enter_context(tc.tile_pool(name="persist", bufs=1))
        ptp = ctx.enter_context(tc.tile_pool(name="ptp", bufs=4))
        work = ctx.enter_context(tc.tile_pool(name="work", bufs=3))
        outp = ctx.enter_context(tc.tile_pool(name="outp", bufs=4))
        psa = ctx.enter_context(tc.tile_pool(name="psa", bufs=2, space="PSUM"))
        ps = ctx.enter_context(tc.tile_pool(name="ps", bufs=6, space="PSUM"))

        # ---- constants ----
        xt_s = consts.tile([128, NO, S], BF16, tag="xts")
        wq_s = consts.tile([128, NO, CW], BF16, tag="wq")
        wk_s = consts.tile([128, NO, CW], BF16, tag="wk")
        wv_s = consts.tile([128, NO, CW], BF16, tag="wv")
        wo_s = consts.tile([128, HPC, D], BF16, tag="wo")
        rc = consts.tile([DH, S], BF16, tag="rc")      # cos, both halves
        rs = consts.tile([DH, S], BF16, tag="rs")      # sin, sign-folded
        tri = consts.tile([128, 128], BF16, tag="tri")  # lower-tri mask
        on16 = consts.tile([128, 1], BF16, tag="on16")
        rcu = consts.tile([DH, W], F32, tag="rcu")
        rsu = consts.tile([DH, W], F32, tag="rsu")
        rc512 = consts.tile([DH, NCH], F32, tag="rc512")
        rs512 = consts.tile([DH, NCH], F32, tag="rs512")
        for o in range(NO):
            sl = slice(128 * o, 128 * (o + 1))
            nc.scalar.dma_start(wv_s[:, o, :], wv_d[sl, :])
            nc.scalar.dma_start(wq_s[:, o, :], wq_d[sl, :])
            nc.scalar.dma_start(wk_s[:, o, :], wk_d[sl, :])
            nc.sync.dma_start(xt_s[:, o, :], xt_d[sl, :])
            if o == 0:
                nc.scalar.dma_start(rcu[:], rcu_d)
                nc.scalar.dma_start(rsu[:], rsu_d)
                nc.scalar.dma_start(rc512[:], rc512_d)
                nc.scalar.dma_start(rs512[:], rs512_d)
        # rope tables over full S via the angle-addition identity
        for ci in range(NCH):
            sl = slice(W * ci, W * (ci + 1))
            tm = work.tile([DH, W], F32, tag="t1")
            nc.vector.tensor_scalar_mul(tm[:], rsu[:], rs512[:, ci:ci + 1])
            nc.vector.scalar_tensor_tensor(
                rc[:, sl], rcu[:], rc512[:, ci:ci + 1], tm[:],
                mybir.AluOpType.mult, mybir.AluOpType.subtract,
            )
            tm2 = work.tile([DH, W], F32, tag="t2")
            nc.vector.tensor_scalar_mul(tm2[:], rcu[:], rs512[:, ci:ci + 1])
            nc.vector.scalar_tensor_tensor(
                rs[:, sl], rsu[:], rc512[:, ci:ci + 1], tm2[:],
                mybir.AluOpType.mult, mybir.AluOpType.add,
            )
        iot = work.tile([128, 128], F32, tag="t1")
        nc.gpsimd.iota(
            iot[:], pattern=[[1, 128]], base=0, channel_multiplier=-1,
            allow_small_or_imprecise_dtypes=True,
        )
        nc.vector.tensor_scalar(
            tri[:], iot[:], 0.0, None, mybir.AluOpType.is_ge,
        )
        nc.gpsimd.memset(on16[:], 1.0)
        nc.scalar.dma_start(wo_s[:], wo_d.rearrange("(h p) n -> p h n", p=128))

        rep_ctx = (
            tc.For_i(
                0, reps, 1,
                hint_engines=tuple(
                    getattr(mybir.EngineType, e)
                    for e in ("PE", "DVE", "Activation", "SP", "Pool")
                ),
            )
            if reps > 1 else nullcontext()
        )
        ctx.enter_context(rep_ctx)

        # ---- persistent per-rep tensors ----
        qT = [persist.tile([DH, S], BF16, tag=f"qT{h}", name=f"qT{h}")
              for h in range(HPC)]
        kT = [persist.tile([DH, S], BF16, tag=f"kT{h}", name=f"kT{h}")
              for h in range(HPC)]
        v16 = persist.tile([128, NB, CW], BF16, tag="v16")
        aT = persist.tile([128, HPC, S], BF16, tag="aT")

        def rope_apply(psum, dst_slice, s0, w):
            # dst[0:64]   = raw[0:64]*cos - raw[64:128]*sin
            # dst[64:128] = raw[64:128]*cos + raw[0:64]*sin
            # rs carries the sign split (-sin top half, +sin bottom half).
            cs = rc[:, s0:s0 + w]
            sn = rs[:, s0:s0 + w]
            raw = work.tile([128, W], BF16, tag="raw")
            nc.scalar.copy(raw[:, :w], psum[:])          # frees the PSUM bank
            t1 = work.tile([128, W], BF16, tag="t1")
            nc.vector.tensor_mul(t1[:, :w], raw[:, :w], cs)
            tsw = work.tile([128, W], BF16, tag="t2")
            nc.vector.tensor_copy(tsw[0:64, :w], raw[64:128, :w])
            nc.vector.tensor_copy(tsw[64:128, :w], raw[0:64, :w])
            nc.vector.tensor_mul(tsw[:, :w], tsw[:, :w], sn)
            nc.vector.tensor_add(dst_slice, t1[:, :w], tsw[:, :w])

        def stage_a(ci):
            s0 = W * ci
            for h in range(HPC):
                pq = psa.tile([128, W], F32, tag="psa", name=f"pq{ci}_{h}")
                pk = psa.tile([128, W], F32, tag="psa", name=f"pk{ci}_{h}")
                for o in range(NO):
                    st = dict(start=(o == 0), stop=(o == NO - 1))
                    nc.tensor.matmul(
                        pq[:], wq_s[:, o, 128 * h:128 * (h + 1)],
                        xt_s[:, o, s0:s0 + W], **st
                    )
                    nc.tensor.matmul(
                        pk[:], wk_s[:, o, 128 * h:128 * (h + 1)],
                        xt_s[:, o, s0:s0 + W], **st
                    )
                rope_apply(pq, qT[h][:, s0:s0 + W], s0, W)
                rope_apply(pk, kT[h][:, s0:s0 + W], s0, W)
            for m4 in range(4):
                pv = psa.tile([128, CW], F32, tag="psa", name=f"pv{ci}_{m4}")
                for o in range(NO):
                    nc.tensor.matmul(
                        pv[:],
                        xt_s[:, o, s0 + 128 * m4:s0 + 128 * (m4 + 1)],
                        wv_s[:, o, :],
                        start=(o == 0), stop=(o == NO - 1),
                    )
                nc.vector.tensor_copy(v16[:, 4 * ci + m4, :], pv[:])

        def stage_b(ci):
            # both heads interleaved block-wise: QK(h1) hides exp(h0) latency
            s0 = W * ci
            sb0 = s0 // 128
            nb = sb0 + 4
            po = [ps.tile([128, W], F32, tag="ps", name=f"po{h}_{ci}")
                  for h in range(HPC)]
            pd = [ps.tile([1, W], F32, tag="ps", name=f"pd{h}_{ci}")
                  for h in range(HPC)]
            for b in range(nb):
                r = b - sb0
                c0 = 128 * r if r > 0 else 0
                stv = dict(start=(b == 0), stop=(b == nb - 1))
                pts = []
                for h in range(HPC):
                    pss = ps.tile([128, W], F32, tag="ps",
                                  name=f"pss{h}_{ci}")
                    nc.tensor.matmul(
                        pss[:, c0:],
                        kT[h][:, 128 * b:128 * (b + 1)],
                        qT[h][:, s0 + c0:s0 + W],
                        start=True, stop=True,
                    )
                    pt = ptp.tile([128, W], BF16, tag="pt")
                    nc.scalar.activation(
                        pt[:, c0:], pss[:, c0:],
                        mybir.ActivationFunctionType.Exp, scale=SCALE,
                    )
                    if r >= 0:
                        nc.vector.tensor_mul(
                            pt[:, c0:c0 + 128], pt[:, c0:c0 + 128], tri[:]
                        )
                    pts.append(pt)
                for h in range(HPC):
                    nc.tensor.matmul(
                        pd[h][:, c0:], on16[:], pts[h][:, c0:], **stv
                    )
                    nc.tensor.matmul(
                        po[h][:, c0:],
                        v16[:, b, 128 * h:128 * (h + 1)], pts[h][:, c0:],
                        **stv
                    )
            for h in range(HPC):
                rec = work.tile([1, W], F32, tag="rec", bufs=1)
                with nc.allow_low_precision("softmax denom recip"):
                    nc.vector.reciprocal(rec[:], pd[h][:])
                bc = work.tile([128, W], F32, tag="bc")
                nc.gpsimd.partition_broadcast(bc[:], rec[:])
                nc.vector.tensor_mul(aT[:, h, s0:s0 + W], po[h][:], bc[:])

        def stage_c(ci):
            for m4 in range(4):
                m = 4 * ci + m4
                ot = outp.tile([128, D], F16, tag="ot")
                for e in range(4):
                    pf = ps.tile([128, W], F32, tag="ps", name=f"pf{ci}_{m4}")
                    for h2 in range(HPC):
                        nc.tensor.matmul(
                            pf[:],
                            aT[:, h2, 128 * m:128 * (m + 1)],
                            wo_s[:, h2, W * e:W * (e + 1)],
                            start=(h2 == 0),
                            stop=(h2 == HPC - 1),
                        )
                    if e % 2 == 0:
                        nc.vector.tensor_copy(ot[:, W * e:W * (e + 1)], pf[:])
                    else:
                        nc.scalar.copy(ot[:, W * e:W * (e + 1)], pf[:])
                eng = nc.scalar if m % 2 == 0 else nc.sync
                eng.dma_start(out_d[128 * m:128 * (m + 1), :], ot[:])

        for _ in range(pyreps):
            if "A" in stages:
                stage_a(0)
            for ci in range(NCH):
                if "B" in stages:
                    stage_b(ci)
                if "A" in stages and ci + 1 < NCH:
                    stage_a(ci + 1)
                if "C" in stages:
                    stage_c(ci)

    nc.compile()
    return nc


def _host_inputs(x, Wq, Wk, Wv, Wo):
    x = np.asarray(x, dtype=np.float32).reshape(S, D)
    xt16 = np.ascontiguousarray(x.T).astype(ml_dtypes.bfloat16)

    half = DH // 2
    inv_freq = (
        ROT_FACTOR
        / (ROPE_BASE ** (np.arange(0, half, dtype=np.float32) * 2.0 / DH))
    ).astype(np.float32)
    sgn = np.where(np.arange(DH) < half, -1.0, 1.0).astype(np.float32)[:, None]
    fd = np.concatenate([inv_freq, inv_freq]).astype(np.float32)[:, None]
    u = np.arange(W, dtype=np.float32)[None, :]
    jj = (float(W) * np.arange(NCH, dtype=np.float32))[None, :]
    ropecu = np.cos(fd * u).astype(np.float32)            # [128, 512]
    ropesu = (sgn * np.sin(fd * u)).astype(np.float32)
    ropec512 = np.cos(fd * jj).astype(np.float32)         # [128, 4]
    ropes512 = (sgn * np.sin(fd * jj)).astype(np.float32)

    Wq = np.asarray(Wq, dtype=np.float32)
    Wk = np.asarray(Wk, dtype=np.float32)
    Wv = np.asarray(Wv, dtype=np.float32)
    Wo = np.asarray(Wo, dtype=np.float32)

    in_maps = []
    for c in range(NCORES):
        sl = slice(CW * c, CW * (c + 1))
        in_maps.append(
            {
                "xt16": xt16,
                "wq16": np.ascontiguousarray(Wq[:, sl]).astype(ml_dtypes.bfloat16),
                "wk16": np.ascontiguousarray(Wk[:, sl]).astype(ml_dtypes.bfloat16),
                "wv16": np.ascontiguousarray(Wv[:, sl]).astype(ml_dtypes.bfloat16),
                "wo16": np.ascontiguousarray(Wo[sl, :]).astype(ml_dtypes.bfloat16),
                "ropecu": ropecu,
                "ropesu": ropesu,
                "ropec512": ropec512,
                "ropes512": ropes512,
            }
        )
    return in_maps


_NC_CACHE = None


def kernel(x, Wq, Wk, Wv, Wo):
    global _NC_CACHE
    if _NC_CACHE is None:
        _NC_CACHE = _build_nc()
    in_maps = _host_inputs(x, Wq, Wk, Wv, Wo)
    res = run_bass_kernel_spmd(_NC_CACHE, in_maps, core_ids=list(range(NCORES)))
    out = np.zeros((S, D), dtype=np.float32)
    for r in res.results:
        out += r["out"].astype(np.float32)
    return out.reshape(1, S, D)


# revision 19
# speedup vs baseline: 1.0560x; 1.0560x over previous
"""PhiHarmonicAttention (B=1, S=2048, D=2048, H=16, Dh=128) on 8 Trainium2 cores.

Sharding: tensor-parallel over heads - 2 heads per core.
  - Wq/Wk/Wv column-sliced (256 cols per core), Wo row-sliced (256 rows).
  - Each core computes q^T/k^T (RoPE'd) + v for its 2 heads, causal
    softmax(QK^T)V in transposed layout, and a partial x-out product with its
    Wo slice. Host sums the 8 partials (TP row-parallel reduction).

All matmuls run in bf16 (fp32 PSUM accumulate): ~0.3% rms per GEMM, well
inside the harness 2e-2 gate, and halves HBM traffic vs f32r. fp8 was
measured (in CoreSim) at ~8% output error - attention is a weighted mean
of zero-mean vectors, so per-element quantization noise does NOT average
away relative to the signal - hence bf16 everywhere.

Pipeline: per 512-wide seq chunk ci - projections A(ci), then attention
B(h0,ci), B(h1,ci), then output projection C(ci). Causality means B(*,ci)
only needs A(0..ci), so PE never starves at stage boundaries.

Engine budget: PE does all GEMMs; ACT does exp (+half the out copies);
DVE does rope muls, masking, softmax normalize (+half the out copies);
Pool (gpsimd) takes the PSUM drains (rope raw copy, v copy) and the
denominator partition-broadcast.
"""
import numpy as np
import ml_dtypes
from contextlib import ExitStack, nullcontext

import concourse.bass as bass
import concourse.tile as tile
from concourse import bacc, mybir
from concourse.bass_utils import run_bass_kernel_spmd

S = 2048
D = 2048
H = 16
DH = 128
NCORES = 8
HPC = H // NCORES          # heads per core = 2
CW = HPC * DH              # weight col-slice per core = 256
NO = D // 128              # contraction chunks = 16
W = 512                    # seq chunk width
NCH = S // W               # seq chunks = 4
NB = S // 128              # seq blocks = 16
SCALE = float(1.0 / np.sqrt(np.float32(DH)))

ROT_FACTOR = (1.0 + 5.0 ** 0.5) / 2.0 - 1.0
ROPE_BASE = 10000.0

F32 = mybir.dt.float32
BF16 = mybir.dt.bfloat16
F16 = mybir.dt.float16


def _build_nc(reps=1, stages="ABC", pyreps=1):
    nc = bacc.Bacc("TRN2", target_bir_lowering=False, debug=False, num_devices=NCORES)

    xt_d = nc.dram_tensor("xt16", [D, S], BF16, kind="ExternalInput").ap()
    wq_d = nc.dram_tensor("wq16", [D, CW], BF16, kind="ExternalInput").ap()
    wk_d = nc.dram_tensor("wk16", [D, CW], BF16, kind="ExternalInput").ap()
    wv_d = nc.dram_tensor("wv16", [D, CW], BF16, kind="ExternalInput").ap()
    wo_d = nc.dram_tensor("wo16", [CW, D], BF16, kind="ExternalInput").ap()
    rcu_d = nc.dram_tensor("ropecu", [DH, W], F32, kind="ExternalInput").ap()
    rsu_d = nc.dram_tensor("ropesu", [DH, W], F32, kind="ExternalInput").ap()
    rc512_d = nc.dram_tensor("ropec512", [DH, NCH], F32, kind="ExternalInput").ap()
    rs512_d = nc.dram_tensor("ropes512", [DH, NCH], F32, kind="ExternalInput").ap()
    out_d = nc.dram_tensor("out", [S, D], F16, kind="ExternalOutput").ap()

    with ExitStack() as ctx:
        tc = ctx.enter_context(tile.TileContext(nc))
        consts = ctx.enter_context(tc.tile_pool(name="consts", bufs=1))
        persist = ctx.enter_context(tc.tile_pool(name="persist", bufs=1))
        ptp = ctx.enter_context(tc.tile_pool(name="ptp", bufs=6))
        work = ctx.enter_context(tc.tile_pool(name="work", bufs=3))
        outp = ctx.enter_context(tc.tile_pool(name="outp", bufs=4))
        psa = ctx.enter_context(tc.tile_pool(name="psa", bufs=2, space="PSUM"))
        ps = ctx.enter_context(tc.tile_pool(name="ps", bufs=6, space="PSUM"))

        # ---- constants ----
        xt_s = consts.tile([128, NO, S], BF16, tag="xts")
        wq_s = consts.tile([128, NO, CW], BF16, tag="wq")
        wk_s = consts.tile([128, NO, CW], BF16, tag="wk")
        wv_s = consts.tile([128, NO, CW], BF16, tag="wv")
        wo_s = consts.tile([128, HPC, D], BF16, tag="wo")
        rc = consts.tile([DH, S], BF16, tag="rc")      # cos, both halves
        rs = consts.tile([DH, S], BF16, tag="rs")      # sin, sign-folded
        tri = consts.tile([128, 128], BF16, tag="tri")  # lower-tri mask
        on16 = consts.tile([128, 1], BF16, tag="on16")
        rcu = consts.tile([DH, W], F32, tag="rcu")
        rsu = consts.tile([DH, W], F32, tag="rsu")
        rc512 = consts.tile([DH, NCH], F32, tag="rc512")
        rs512 = consts.tile([DH, NCH], F32, tag="rs512")
        for o in range(NO):
            sl = slice(128 * o, 128 * (o + 1))
            nc.scalar.dma_start(wv_s[:, o, :], wv_d[sl, :])
            nc.scalar.dma_start(wq_s[:, o, :], wq_d[sl, :])
            nc.scalar.dma_start(wk_s[:, o, :], wk_d[sl, :])
            nc.sync.dma_start(xt_s[:, o, :], xt_d[sl, :])
            if o == 0:
                nc.scalar.dma_start(rcu[:], rcu_d)
                nc.scalar.dma_start(rsu[:], rsu_d)
                nc.scalar.dma_start(rc512[:], rc512_d)
                nc.scalar.dma_start(rs512[:], rs512_d)
        # rope tables over full S via the angle-addition identity
        for ci in range(NCH):
            sl = slice(W * ci, W * (ci + 1))
            tm = work.tile([DH, W], F32, tag="t1")
            nc.vector.tensor_scalar_mul(tm[:], rsu[:], rs512[:, ci:ci + 1])
            nc.vector.scalar_tensor_tensor(
                rc[:, sl], rcu[:], rc512[:, ci:ci + 1], tm[:],
                mybir.AluOpType.mult, mybir.AluOpType.subtract,
            )
            tm2 = work.tile([DH, W], F32, tag="t2")
            nc.vector.tensor_scalar_mul(tm2[:], rcu[:], rs512[:, ci:ci + 1])
            nc.vector.scalar_tensor_tensor(
                rs[:, sl], rsu[:], rc512[:, ci:ci + 1], tm2[:],
                mybir.AluOpType.mult, mybir.AluOpType.add,
            )
        iot = work.tile([128, 128], F32, tag="t1")
        nc.gpsimd.iota(
            iot[:], pattern=[[1, 128]], base=0, channel_multiplier=-1,
            allow_small_or_imprecise_dtypes=True,
        )
        nc.vector.tensor_scalar(
            tri[:], iot[:], 0.0, None, mybir.AluOpType.is_ge,
        )
        nc.gpsimd.memset(on16[:], 1.0)
        nc.scalar.dma_start(wo_s[:], wo_d.rearrange("(h p) n -> p h n", p=128))

        rep_ctx = (
            tc.For_i(
                0, reps, 1,
                hint_engines=tuple(
                    getattr(mybir.EngineType, e)
                    for e in ("PE", "DVE", "Activation", "SP", "Pool")
                ),
            )
            if reps > 1 else nullcontext()
        )
        ctx.enter_context(rep_ctx)

        # ---- persistent per-rep tensors ----
        qT = [persist.tile([DH, S], BF16, tag=f"qT{h}", name=f"qT{h}")
              for h in range(HPC)]
        kT = [persist.tile([DH, S], BF16, tag=f"kT{h}", name=f"kT{h}")
              for h in range(HPC)]
        v16 = persist.tile([128, NB, CW], BF16, tag="v16")
        aT = persist.tile([128, HPC, S], BF16, tag="aT")

        def rope_apply(psum, dst_slice, s0, w):
            # dst[0:64]   = raw[0:64]*cos - raw[64:128]*sin
            # dst[64:128] = raw[64:128]*cos + raw[0:64]*sin
            # rs carries the sign split (-sin top half, +sin bottom half).
            cs = rc[:, s0:s0 + w]
            sn = rs[:, s0:s0 + w]
            raw = work.tile([128, W], BF16, tag="raw")
            nc.scalar.copy(raw[:, :w], psum[:])          # frees the PSUM bank
            t1 = work.tile([128, W], BF16, tag="t1")
            nc.vector.tensor_mul(t1[:, :w], raw[:, :w], cs)
            tsw = work.tile([128, W], BF16, tag="t2")
            nc.vector.tensor_copy(tsw[0:64, :w], raw[64:128, :w])
            nc.vector.tensor_copy(tsw[64:128, :w], raw[0:64, :w])
            nc.vector.tensor_mul(tsw[:, :w], tsw[:, :w], sn)
            nc.vector.tensor_add(dst_slice, t1[:, :w], tsw[:, :w])

        def stage_a(ci):
            s0 = W * ci
            for h in range(HPC):
                pq = psa.tile([128, W], F32, tag="psa", name=f"pq{ci}_{h}")
                pk = psa.tile([128, W], F32, tag="psa", name=f"pk{ci}_{h}")
                for o in range(NO):
                    st = dict(start=(o == 0), stop=(o == NO - 1))
                    nc.tensor.matmul(
                        pq[:], wq_s[:, o, 128 * h:128 * (h + 1)],
                        xt_s[:, o, s0:s0 + W], **st
                    )
                    nc.tensor.matmul(
                        pk[:], wk_s[:, o, 128 * h:128 * (h + 1)],
                        xt_s[:, o, s0:s0 + W], **st
                    )
                rope_apply(pq, qT[h][:, s0:s0 + W], s0, W)
                rope_apply(pk, kT[h][:, s0:s0 + W], s0, W)
            for m4 in range(4):
                pv = psa.tile([128, CW], F32, tag="psa", name=f"pv{ci}_{m4}")
                for o in range(NO):
                    nc.tensor.matmul(
                        pv[:],
                        xt_s[:, o, s0 + 128 * m4:s0 + 128 * (m4 + 1)],
                        wv_s[:, o, :],
                        start=(o == 0), stop=(o == NO - 1),
                    )
                nc.vector.tensor_copy(v16[:, 4 * ci + m4, :], pv[:])

        def stage_b(ci):
            # both heads interleaved block-wise: QK(h1) hides exp(h0) latency
            s0 = W * ci
            sb0 = s0 // 128
            nb = sb0 + 4
            po = [ps.tile([128, W], F32, tag="ps", name=f"po{h}_{ci}")
                  for h in range(HPC)]
            pd = [ps.tile([1, W], F32, tag="ps", name=f"pd{h}_{ci}")
                  for h in range(HPC)]
            # software pipeline: QK/exp run LAG blocks ahead of pd/po so the
            # in-order PE stream never waits on the exp chain
            LAG = 2
            ptq = {}
            for bb in range(nb + LAG):
                if bb < nb:
                    b = bb
                    r = b - sb0
                    c0 = 128 * r if r > 0 else 0
                    pts = []
                    for h in range(HPC):
                        pss = ps.tile([128, W], F32, tag="ps",
                                      name=f"pss{h}_{ci}")
                        nc.tensor.matmul(
                            pss[:, c0:],
                            kT[h][:, 128 * b:128 * (b + 1)],
                            qT[h][:, s0 + c0:s0 + W],
                            start=True, stop=True,
                        )
                        pt = ptp.tile([128, W], BF16, tag="pt")
                        nc.scalar.activation(
                            pt[:, c0:], pss[:, c0:],
                            mybir.ActivationFunctionType.Exp, scale=SCALE,
                        )
                        if r >= 0:
                            nc.vector.tensor_mul(
                                pt[:, c0:c0 + 128], pt[:, c0:c0 + 128],
                                tri[:]
                            )
                        pts.append(pt)
                    ptq[b] = pts
                if bb >= LAG:
                    b = bb - LAG
                    r = b - sb0
                    c0 = 128 * r if r > 0 else 0
                    stv = dict(start=(b == 0), stop=(b == nb - 1))
                    pts = ptq.pop(b)
                    for h in range(HPC):
                        nc.tensor.matmul(
                            pd[h][:, c0:], on16[:], pts[h][:, c0:], **stv
                        )
                        nc.tensor.matmul(
                            po[h][:, c0:],
                            v16[:, b, 128 * h:128 * (h + 1)],
                            pts[h][:, c0:], **stv
                        )
            for h in range(HPC):
                rec = work.tile([1, W], F32, tag="rec", bufs=1)
                with nc.allow_low_precision("softmax denom recip"):
                    nc.vector.reciprocal(rec[:], pd[h][:])
                bc = work.tile([128, W], F32, tag="bc")
                nc.gpsimd.partition_broadcast(bc[:], rec[:])
                nc.vector.tensor_mul(aT[:, h, s0:s0 + W], po[h][:], bc[:])

        def stage_c(ci):
            for m4 in range(4):
                m = 4 * ci + m4
                ot = outp.tile([128, D], F16, tag="ot")
                for e in range(4):
                    pf = ps.tile([128, W], F32, tag="ps", name=f"pf{ci}_{m4}")
                    for h2 in range(HPC):
                        nc.tensor.matmul(
                            pf[:],
                            aT[:, h2, 128 * m:128 * (m + 1)],
                            wo_s[:, h2, W * e:W * (e + 1)],
                            start=(h2 == 0),
                            stop=(h2 == HPC - 1),
                        )
                    if e % 2 == 0:
                        nc.vector.tensor_copy(ot[:, W * e:W * (e + 1)], pf[:])
                    else:
                        nc.scalar.copy(ot[:, W * e:W * (e + 1)], pf[:])
                eng = nc.scalar if m % 2 == 0 else nc.sync
                eng.dma_start(out_d[128 * m:128 * (m + 1), :], ot[:])

        for _ in range(pyreps):
            if "A" in stages:
                stage_a(0)
            for ci in range(NCH):
                if "B" in stages:
                    stage_b(ci)
                if "A" in stages and ci + 1 < NCH:
                    stage_a(ci + 1)
                if "C" in stages:
                    stage_c(ci)

    nc.compile()
    return nc


def _host_inputs(x, Wq, Wk, Wv, Wo):
    x = np.asarray(x, dtype=np.float32).reshape(S, D)
    xt16 = np.ascontiguousarray(x.T).astype(ml_dtypes.bfloat16)

    half = DH // 2
    inv_freq = (
        ROT_FACTOR
        / (ROPE_BASE ** (np.arange(0, half, dtype=np.float32) * 2.0 / DH))
    ).astype(np.float32)
    sgn = np.where(np.arange(DH) < half, -1.0, 1.0).astype(np.float32)[:, None]
    fd = np.concatenate([inv_freq, inv_freq]).astype(np.float32)[:, None]
    u = np.arange(W, dtype=np.float32)[None, :]
    jj = (float(W) * np.arange(NCH, dtype=np.float32))[None, :]
    ropecu = np.cos(fd * u).astype(np.float32)            # [128, 512]
    ropesu = (sgn * np.sin(fd * u)).astype(np.float32)
    ropec512 = np.cos(fd * jj).astype(np.float32)         # [128, 4]
    ropes512 = (sgn * np.sin(fd * jj)).astype(np.float32)

    Wq = np.asarray(Wq, dtype=np.float32)
    Wk = np.asarray(Wk, dtype=np.float32)
    Wv = np.asarray(Wv, dtype=np.float32)
    Wo = np.asarray(Wo, dtype=np.float32)

    in_maps = []
    for c in range(NCORES):
        sl = slice(CW * c, CW * (c + 1))
        in_maps.append(
            {
                "xt16": xt16,
                "wq16": np.ascontiguousarray(Wq[:, sl]).astype(ml_dtypes.bfloat16),
                "wk16": np.ascontiguousarray(Wk[:, sl]).astype(ml_dtypes.bfloat16),
                "wv16": np.ascontiguousarray(Wv[:, sl]).astype(ml_dtypes.bfloat16),
                "wo16": np.ascontiguousarray(Wo[sl, :]).astype(ml_dtypes.bfloat16),
                "ropecu": ropecu,
                "ropesu": ropesu,
                "ropec512": ropec512,
                "ropes512": ropes512,
            }
        )
    return in_maps


_NC_CACHE = None


def kernel(x, Wq, Wk, Wv, Wo):
    global _NC_CACHE
    if _NC_CACHE is None:
        _NC_CACHE = _build_nc()
    in_maps = _host_inputs(x, Wq, Wk, Wv, Wo)
    res = run_bass_kernel_spmd(_NC_CACHE, in_maps, core_ids=list(range(NCORES)))
    out = np.zeros((S, D), dtype=np.float32)
    for r in res.results:
        out += r["out"].astype(np.float32)
    return out.reshape(1, S, D)


# revision 20
# speedup vs baseline: 1.0786x; 1.0214x over previous
"""PhiHarmonicAttention (B=1, S=2048, D=2048, H=16, Dh=128) on 8 Trainium2 cores.

Sharding: tensor-parallel over heads - 2 heads per core.
  - Wq/Wk/Wv column-sliced (256 cols per core), Wo row-sliced (256 rows).
  - Each core computes q^T/k^T (RoPE'd) + v for its 2 heads, causal
    softmax(QK^T)V in transposed layout, and a partial x-out product with its
    Wo slice. Host sums the 8 partials (TP row-parallel reduction).

All matmuls run in bf16 (fp32 PSUM accumulate): ~0.3% rms per GEMM, well
inside the harness 2e-2 gate, and halves HBM traffic vs f32r. fp8 was
measured (in CoreSim) at ~8% output error - attention is a weighted mean
of zero-mean vectors, so per-element quantization noise does NOT average
away relative to the signal - hence bf16 everywhere.

Pipeline: per 512-wide seq chunk ci - projections A(ci), then attention
B(h0,ci), B(h1,ci), then output projection C(ci). Causality means B(*,ci)
only needs A(0..ci), so PE never starves at stage boundaries.

Engine budget: PE does all GEMMs; ACT does exp (+half the out copies);
DVE does rope muls, masking, softmax normalize (+half the out copies);
Pool (gpsimd) takes the PSUM drains (rope raw copy, v copy) and the
denominator partition-broadcast.
"""
import numpy as np
import ml_dtypes
from contextlib import ExitStack, nullcontext

import concourse.bass as bass
import concourse.tile as tile
from concourse import bacc, mybir
from concourse.bass_utils import run_bass_kernel_spmd

S = 2048
D = 2048
H = 16
DH = 128
NCORES = 8
HPC = H // NCORES          # heads per core = 2
CW = HPC * DH              # weight col-slice per core = 256
NO = D // 128              # contraction chunks = 16
W = 512                    # seq chunk width
NCH = S // W               # seq chunks = 4
NB = S // 128              # seq blocks = 16
SCALE = float(1.0 / np.sqrt(np.float32(DH)))

ROT_FACTOR = (1.0 + 5.0 ** 0.5) / 2.0 - 1.0
ROPE_BASE = 10000.0

F32 = mybir.dt.float32
BF16 = mybir.dt.bfloat16
F16 = mybir.dt.float16


def _build_nc(reps=1, stages="ABC", pyreps=1):
    nc = bacc.Bacc("TRN2", target_bir_lowering=False, debug=False, num_devices=NCORES)

    xt_d = nc.dram_tensor("xt16", [D, S], BF16, kind="ExternalInput").ap()
    wq_d = nc.dram_tensor("wq16", [D, CW], BF16, kind="ExternalInput").ap()
    wk_d = nc.dram_tensor("wk16", [D, CW], BF16, kind="ExternalInput").ap()
    wv_d = nc.dram_tensor("wv16", [D, CW], BF16, kind="ExternalInput").ap()
    wo_d = nc.dram_tensor("wo16", [CW, D], BF16, kind="ExternalInput").ap()
    rcu_d = nc.dram_tensor("ropecu", [DH, W], F32, kind="ExternalInput").ap()
    rsu_d = nc.dram_tensor("ropesu", [DH, W], F32, kind="ExternalInput").ap()
    rc512_d = nc.dram_tensor("ropec512", [DH, NCH], F32, kind="ExternalInput").ap()
    rs512_d = nc.dram_tensor("ropes512", [DH, NCH], F32, kind="ExternalInput").ap()
    out_d = nc.dram_tensor("out", [S, D], F16, kind="ExternalOutput").ap()

    with ExitStack() as ctx:
        tc = ctx.enter_context(tile.TileContext(nc))
        consts = ctx.enter_context(tc.tile_pool(name="consts", bufs=1))
        persist = ctx.enter_context(tc.tile_pool(name="persist", bufs=1))
        ptp = ctx.enter_context(tc.tile_pool(name="ptp", bufs=8))
        work = ctx.enter_context(tc.tile_pool(name="work", bufs=3))
        outp = ctx.enter_context(tc.tile_pool(name="outp", bufs=4))
        psa = ctx.enter_context(tc.tile_pool(name="psa", bufs=2, space="PSUM"))
        ps = ctx.enter_context(tc.tile_pool(name="ps", bufs=6, space="PSUM"))

        # ---- constants ----
        xt_s = consts.tile([128, NO, S], BF16, tag="xts")
        wq_s = consts.tile([128, NO, CW], BF16, tag="wq")
        wk_s = consts.tile([128, NO, CW], BF16, tag="wk")
        wv_s = consts.tile([128, NO, CW], BF16, tag="wv")
        wo_s = consts.tile([128, HPC, D], BF16, tag="wo")
        rc = consts.tile([DH, S], BF16, tag="rc")      # cos, both halves
        rs = consts.tile([DH, S], BF16, tag="rs")      # sin, sign-folded
        tri = consts.tile([128, 128], BF16, tag="tri")  # lower-tri mask
        on16 = consts.tile([128, 1], BF16, tag="on16")
        rcu = consts.tile([DH, W], F32, tag="rcu")
        rsu = consts.tile([DH, W], F32, tag="rsu")
        rc512 = consts.tile([DH, NCH], F32, tag="rc512")
        rs512 = consts.tile([DH, NCH], F32, tag="rs512")
        for o in range(NO):
            sl = slice(128 * o, 128 * (o + 1))
            nc.scalar.dma_start(wv_s[:, o, :], wv_d[sl, :])
            nc.scalar.dma_start(wq_s[:, o, :], wq_d[sl, :])
            nc.scalar.dma_start(wk_s[:, o, :], wk_d[sl, :])
            nc.sync.dma_start(xt_s[:, o, :], xt_d[sl, :])
            if o == 0:
                nc.scalar.dma_start(rcu[:], rcu_d)
                nc.scalar.dma_start(rsu[:], rsu_d)
                nc.scalar.dma_start(rc512[:], rc512_d)
                nc.scalar.dma_start(rs512[:], rs512_d)
        # rope tables over full S via the angle-addition identity
        for ci in range(NCH):
            sl = slice(W * ci, W * (ci + 1))
            tm = work.tile([DH, W], F32, tag="t1")
            nc.vector.tensor_scalar_mul(tm[:], rsu[:], rs512[:, ci:ci + 1])
            nc.vector.scalar_tensor_tensor(
                rc[:, sl], rcu[:], rc512[:, ci:ci + 1], tm[:],
                mybir.AluOpType.mult, mybir.AluOpType.subtract,
            )
            tm2 = work.tile([DH, W], F32, tag="t2")
            nc.vector.tensor_scalar_mul(tm2[:], rcu[:], rs512[:, ci:ci + 1])
            nc.vector.scalar_tensor_tensor(
                rs[:, sl], rsu[:], rc512[:, ci:ci + 1], tm2[:],
                mybir.AluOpType.mult, mybir.AluOpType.add,
            )
        iot = work.tile([128, 128], F32, tag="t1")
        nc.gpsimd.iota(
            iot[:], pattern=[[1, 128]], base=0, channel_multiplier=-1,
            allow_small_or_imprecise_dtypes=True,
        )
        nc.vector.tensor_scalar(
            tri[:], iot[:], 0.0, None, mybir.AluOpType.is_ge,
        )
        nc.gpsimd.memset(on16[:], 1.0)
        nc.scalar.dma_start(wo_s[:], wo_d.rearrange("(h p) n -> p h n", p=128))

        rep_ctx = (
            tc.For_i(
                0, reps, 1,
                hint_engines=tuple(
                    getattr(mybir.EngineType, e)
                    for e in ("PE", "DVE", "Activation", "SP", "Pool")
                ),
            )
            if reps > 1 else nullcontext()
        )
        ctx.enter_context(rep_ctx)

        # ---- persistent per-rep tensors ----
        qT = [persist.tile([DH, S], BF16, tag=f"qT{h}", name=f"qT{h}")
              for h in range(HPC)]
        kT = [persist.tile([DH, S], BF16, tag=f"kT{h}", name=f"kT{h}")
              for h in range(HPC)]
        v16 = persist.tile([128, NB, CW], BF16, tag="v16")
        aT = persist.tile([128, HPC, S], BF16, tag="aT")

        def rope_apply(psum, dst_slice, s0, w):
            # dst[0:64]   = raw[0:64]*cos - raw[64:128]*sin
            # dst[64:128] = raw[64:128]*cos + raw[0:64]*sin
            # rs carries the sign split (-sin top half, +sin bottom half).
            cs = rc[:, s0:s0 + w]
            sn = rs[:, s0:s0 + w]
            raw = work.tile([128, W], BF16, tag="raw")
            nc.scalar.copy(raw[:, :w], psum[:])          # frees the PSUM bank
            t1 = work.tile([128, W], BF16, tag="t1")
            nc.vector.tensor_mul(t1[:, :w], raw[:, :w], cs)
            tsw = work.tile([128, W], BF16, tag="t2")
            nc.vector.tensor_copy(tsw[0:64, :w], raw[64:128, :w])
            nc.vector.tensor_copy(tsw[64:128, :w], raw[0:64, :w])
            nc.vector.tensor_mul(tsw[:, :w], tsw[:, :w], sn)
            nc.vector.tensor_add(dst_slice, t1[:, :w], tsw[:, :w])

        def stage_a(ci):
            s0 = W * ci
            for h in range(HPC):
                pq = psa.tile([128, W], F32, tag="psa", name=f"pq{ci}_{h}")
                pk = psa.tile([128, W], F32, tag="psa", name=f"pk{ci}_{h}")
                for o in range(NO):
                    st = dict(start=(o == 0), stop=(o == NO - 1))
                    nc.tensor.matmul(
                        pq[:], wq_s[:, o, 128 * h:128 * (h + 1)],
                        xt_s[:, o, s0:s0 + W], **st
                    )
                    nc.tensor.matmul(
                        pk[:], wk_s[:, o, 128 * h:128 * (h + 1)],
                        xt_s[:, o, s0:s0 + W], **st
                    )
                rope_apply(pq, qT[h][:, s0:s0 + W], s0, W)
                rope_apply(pk, kT[h][:, s0:s0 + W], s0, W)
            for m4 in range(4):
                pv = psa.tile([128, CW], F32, tag="psa", name=f"pv{ci}_{m4}")
                for o in range(NO):
                    nc.tensor.matmul(
                        pv[:],
                        xt_s[:, o, s0 + 128 * m4:s0 + 128 * (m4 + 1)],
                        wv_s[:, o, :],
                        start=(o == 0), stop=(o == NO - 1),
                    )
                nc.vector.tensor_copy(v16[:, 4 * ci + m4, :], pv[:])

        def stage_b(ci):
            # both heads interleaved block-wise: QK(h1) hides exp(h0) latency
            s0 = W * ci
            sb0 = s0 // 128
            nb = sb0 + 4
            po = [ps.tile([128, W], F32, tag="ps", name=f"po{h}_{ci}")
                  for h in range(HPC)]
            pd = [ps.tile([1, W], F32, tag="ps", name=f"pd{h}_{ci}")
                  for h in range(HPC)]
            # software pipeline: QK/exp run LAG blocks ahead of pd/po so the
            # in-order PE stream never waits on the exp chain
            LAG = 2
            ptq = {}
            for bb in range(nb + LAG):
                if bb < nb:
                    b = bb
                    r = b - sb0
                    c0 = 128 * r if r > 0 else 0
                    pts = []
                    for h in range(HPC):
                        pool_b = ps if b % 2 == 0 else psa
                        pss = pool_b.tile([128, W], F32,
                                          tag="ps" if b % 2 == 0 else "psa",
                                          name=f"pss{h}_{ci}")
                        nc.tensor.matmul(
                            pss[:, c0:],
                            kT[h][:, 128 * b:128 * (b + 1)],
                            qT[h][:, s0 + c0:s0 + W],
                            start=True, stop=True,
                        )
                        pt = ptp.tile([128, W], BF16, tag="pt")
                        nc.scalar.activation(
                            pt[:, c0:], pss[:, c0:],
                            mybir.ActivationFunctionType.Exp, scale=SCALE,
                        )
                        if r >= 0:
                            nc.vector.tensor_mul(
                                pt[:, c0:c0 + 128], pt[:, c0:c0 + 128],
                                tri[:]
                            )
                        pts.append(pt)
                    ptq[b] = pts
                if bb >= LAG:
                    b = bb - LAG
                    r = b - sb0
                    c0 = 128 * r if r > 0 else 0
                    stv = dict(start=(b == 0), stop=(b == nb - 1))
                    pts = ptq.pop(b)
                    for h in range(HPC):
                        nc.tensor.matmul(
                            pd[h][:, c0:], on16[:], pts[h][:, c0:], **stv
                        )
                        nc.tensor.matmul(
                            po[h][:, c0:],
                            v16[:, b, 128 * h:128 * (h + 1)],
                            pts[h][:, c0:], **stv
                        )
            for h in range(HPC):
                rec = work.tile([1, W], F32, tag="rec", bufs=1)
                with nc.allow_low_precision("softmax denom recip"):
                    nc.vector.reciprocal(rec[:], pd[h][:])
                bc = work.tile([128, W], F32, tag="bc")
                nc.gpsimd.partition_broadcast(bc[:], rec[:])
                nc.vector.tensor_mul(aT[:, h, s0:s0 + W], po[h][:], bc[:])

        def stage_c(ci):
            for m4 in range(4):
                m = 4 * ci + m4
                ot = outp.tile([128, D], F16, tag="ot")
                for e in range(4):
                    pf = ps.tile([128, W], F32, tag="ps", name=f"pf{ci}_{m4}")
                    for h2 in range(HPC):
                        nc.tensor.matmul(
                            pf[:],
                            aT[:, h2, 128 * m:128 * (m + 1)],
                            wo_s[:, h2, W * e:W * (e + 1)],
                            start=(h2 == 0),
                            stop=(h2 == HPC - 1),
                        )
                    if e % 2 == 0:
                        nc.vector.tensor_copy(ot[:, W * e:W * (e + 1)], pf[:])
                    else:
                        nc.scalar.copy(ot[:, W * e:W * (e + 1)], pf[:])
                eng = nc.scalar if m % 2 == 0 else nc.sync
                eng.dma_start(out_d[128 * m:128 * (m + 1), :], ot[:])

        for _ in range(pyreps):
            if "A" in stages:
                stage_a(0)
            for ci in range(NCH):
                if "B" in stages:
                    stage_b(ci)
                if "A" in stages and ci + 1 < NCH:
                    stage_a(ci + 1)
                if "C" in stages:
                    stage_c(ci)

    nc.compile()
    return nc


def _host_inputs(x, Wq, Wk, Wv, Wo):
    x = np.asarray(x, dtype=np.float32).reshape(S, D)
    xt16 = np.ascontiguousarray(x.T).astype(ml_dtypes.bfloat16)

    half = DH // 2
    inv_freq = (
        ROT_FACTOR
        / (ROPE_BASE ** (np.arange(0, half, dtype=np.float32) * 2.0 / DH))
    ).astype(np.float32)
    sgn = np.where(np.arange(DH) < half, -1.0, 1.0).astype(np.float32)[:, None]
    fd = np.concatenate([inv_freq, inv_freq]).astype(np.float32)[:, None]
    u = np.arange(W, dtype=np.float32)[None, :]
    jj = (float(W) * np.arange(NCH, dtype=np.float32))[None, :]
    ropecu = np.cos(fd * u).astype(np.float32)            # [128, 512]
    ropesu = (sgn * np.sin(fd * u)).astype(np.float32)
    ropec512 = np.cos(fd * jj).astype(np.float32)         # [128, 4]
    ropes512 = (sgn * np.sin(fd * jj)).astype(np.float32)

    Wq = np.asarray(Wq, dtype=np.float32)
    Wk = np.asarray(Wk, dtype=np.float32)
    Wv = np.asarray(Wv, dtype=np.float32)
    Wo = np.asarray(Wo, dtype=np.float32)

    in_maps = []
    for c in range(NCORES):
        sl = slice(CW * c, CW * (c + 1))
        in_maps.append(
            {
                "xt16": xt16,
                "wq16": np.ascontiguousarray(Wq[:, sl]).astype(ml_dtypes.bfloat16),
                "wk16": np.ascontiguousarray(Wk[:, sl]).astype(ml_dtypes.bfloat16),
                "wv16": np.ascontiguousarray(Wv[:, sl]).astype(ml_dtypes.bfloat16),
                "wo16": np.ascontiguousarray(Wo[sl, :]).astype(ml_dtypes.bfloat16),
                "ropecu": ropecu,
                "ropesu": ropesu,
                "ropec512": ropec512,
                "ropes512": ropes512,
            }
        )
    return in_maps


_NC_CACHE = None


def kernel(x, Wq, Wk, Wv, Wo):
    global _NC_CACHE
    if _NC_CACHE is None:
        _NC_CACHE = _build_nc()
    in_maps = _host_inputs(x, Wq, Wk, Wv, Wo)
    res = run_bass_kernel_spmd(_NC_CACHE, in_maps, core_ids=list(range(NCORES)))
    out = np.zeros((S, D), dtype=np.float32)
    for r in res.results:
        out += r["out"].astype(np.float32)
    return out.reshape(1, S, D)
